# revision 32
# baseline (speedup 1.0000x reference)
"""BertSelfAttention (B=2, S=2048, H=1024, 16 heads x 64) on 8 TRN2 NeuronCores.

Sharding: data parallel on batch (4 cores per batch) x tensor parallel on
heads (4 heads per core). No cross-core comms; each core computes
out[b, :, 256*g:256*(g+1)] for its head group g.

v10 design notes (all measured on HW traces):
- Every DMA around an X-bar transpose serializes on the previous DMA's
  completion (X-bar quiescence), so startup is ONE serial chain on the sync
  queue ordered by consumption: consts | wk+wq | bvs | T(s0-511) | wv |
  T(s512-1023) | T(s1024-2047). Weights are host-pre-arranged so their DMA
  is contiguous per partition.
- Tile deps compile to monotonic per-engine counters: exp(i) waits the PE
  counter at its sc(i) emission, so anything emitted before a sc inflates
  the exp critical path. Deadline projections (dlfill) are the only work
  emitted before sc(i+2); all other fillers go after.
- The PE p-state reaches full clock only after ~3us of continuous busy; a
  discarded warmup matmul chain covers the DMA-chain wait.
- Iteration order interleaves qb0/qb1 first halves so the ACT exp stream
  has work while the PE grinds v-projections, and the k>=8 data (last
  transposes) arrives before anything needs it. qb0/qb1 k0-7 context is
  staged to SBUF f32 and merged in the k8-15 pass.
- Score PSUM pool has 3 slots (lead-2 emission); projection PSUM shares the
  same slots via tag-sharing; output transposes reuse the two ctx banks
  (plus idle score slots on the final drain). 8 PSUM banks exactly.

Per-core pipeline:
  A) hiddenT via 24 serial X-bar transposes ([512,128]x16 + [1024,128]x8)
  B) kT/qT [128(d of pair), 2048(s)] bf16 (1/8 scale + bias folded) via
     256-col groups; V [128(s), 4heads, 65] bf16 with ones column
  C) per (pair, qb, k): scoresT h0|h1 packed -> one exp [128,1024] (mask
     bias) -> bf16 et, ctxT[65, 512] += v_ext.T @ et per head
  D) PE-transpose ctxT (bf16) -> [q, 65], DVE reciprocal + scale, DMA out
     per (pair, qb) half-block during the stream
"""

import ml_dtypes
import numpy as np

import concourse.bass as bass
import concourse.tile as tile
from concourse import bacc, mybir
from concourse.bass_utils import run_bass_kernel_spmd
from concourse.masks import make_identity

F32 = mybir.dt.float32
BF16 = mybir.dt.bfloat16
EXP = mybir.ActivationFunctionType.Exp

B, S, H = 2, 2048, 1024
NH, HD = 16, 64
NCORES = 8
HPC = 4  # heads per core
DPC = HPC * HD  # 256 output dims per core
SC = S // 128  # 16 s/k chunks
JC = H // 128  # 8 contraction chunks
QB = 512  # q block in attention inner loop
NQB = S // QB  # 4
GC = 8  # 256-col projection groups per (w, pair)

# flat iteration order: qb0/qb1 first halves interleaved into the
# v-projection grind, second halves after the late transposes land,
# then qb2/qb3 and pair 1 straight
ITERS = (
    [(0, 0, k) for k in range(8)]
    + [(0, 1, k) for k in range(8)]
    + [(0, 0, k) for k in range(8, SC)]
    + [(0, 1, k) for k in range(8, SC)]
    + [(0, 2, k) for k in range(SC)]
    + [(0, 3, k) for k in range(SC)]
    + [(1, qb, k) for qb in range(NQB) for k in range(SC)]
)
NIT = len(ITERS)  # 128
SPLIT_QBS = {(0, 0), (0, 1)}  # qbs whose k0-7 context is staged and merged


def build():
    nc = bacc.Bacc(
        "TRN2",
        target_bir_lowering=False,
        debug=False,
        enable_asserts=False,
        num_devices=NCORES,
    )
    hidb = nc.dram_tensor("hidb", [S, H], BF16, kind="ExternalInput").ap()
    wkq = nc.dram_tensor("wkq", [128, 2, JC, DPC], BF16, kind="ExternalInput").ap()
    wvd = nc.dram_tensor("wvd", [128, JC, DPC], BF16, kind="ExternalInput").ap()
    # mask | bqs | bks packed into one [128, SC+4] f32 tensor
    consts = nc.dram_tensor("consts", [128, SC + 4], F32, kind="ExternalInput").ap()
    bvs = nc.dram_tensor("bvs", [1, DPC], BF16, kind="ExternalInput").ap()
    out = nc.dram_tensor("out", [S, DPC], F32, kind="ExternalOutput").ap()

    with tile.TileContext(nc) as tc:
        with (
            tc.tile_pool(name="persist", bufs=1) as persist,
            tc.tile_pool(name="etp", bufs=8) as etp,
            tc.tile_pool(name="ctsp", bufs=2) as ctsp,
            tc.tile_pool(name="tpsb", bufs=4) as tpsb,
            tc.tile_pool(name="rcp", bufs=4) as rcp,
            tc.tile_pool(name="scps", bufs=3, space="PSUM") as scps,
            tc.tile_pool(name="ctxps", bufs=1, space="PSUM") as ctxps,
        ):
            # dummy operands for the PE warmup chain
            dummyw = persist.tile([128, 128], BF16, tag="dummyw")
            nc.vector.memset(dummyw[:], 0.0)
            dummy = persist.tile([128, 512], BF16, tag="dummy")
            nc.vector.memset(dummy[:], 0.0)

            # startup DMA chain on the sync queue (see module docstring)
            consts_sb = persist.tile([128, SC + 4], F32, tag="consts")
            nc.sync.dma_start(consts_sb[:], consts)
            mask_sb = consts_sb[:, 0:SC]
            bqs_sb = consts_sb[:, SC : SC + 2]
            bks_sb = consts_sb[:, SC + 2 : SC + 4]
            wkq_t = persist.tile([128, 2, JC, DPC], BF16, tag="wkq", name="w_kq")
            nc.sync.dma_start(wkq_t[:], wkq)
            w_sb = {"wk": wkq_t[:, 0], "wq": wkq_t[:, 1]}
            bvs_sb = persist.tile([1, DPC], BF16, tag="bvs")
            nc.sync.dma_start(bvs_sb[:], bvs)

            hidA = [
                persist.tile([128, 512], BF16, tag=f"hA{j}", name=f"hA{j}")
                for j in range(JC)
            ]
            hidB = [
                persist.tile([128, 512], BF16, tag=f"hB{j}", name=f"hB{j}")
                for j in range(JC)
            ]
            hidC = [
                persist.tile([128, 1024], BF16, tag=f"hC{j}", name=f"hC{j}")
                for j in range(JC)
            ]
            for j in range(JC):
                nc.sync.dma_start_transpose(
                    out=hidA[j][:], in_=hidb[0:512, j * 128 : (j + 1) * 128]
                )
            wv_t = persist.tile([128, JC, DPC], BF16, tag="wv", name="w_wv")
            nc.sync.dma_start(wv_t[:], wvd)
            w_sb["wv"] = wv_t
            for j in range(JC):
                nc.sync.dma_start_transpose(
                    out=hidB[j][:], in_=hidb[512:1024, j * 128 : (j + 1) * 128]
                )
            for j in range(JC):
                nc.sync.dma_start_transpose(
                    out=hidC[j][:], in_=hidb[1024:S, j * 128 : (j + 1) * 128]
                )

            ones1_f = persist.tile([1, 128], F32, tag="ones1f")
            nc.vector.memset(ones1_f[:], 1.0)
            # warm the ACT exp table during startup
            warm = persist.tile([1, 1], F32, tag="warm")
            nc.scalar.activation(warm[:], ones1_f[:, 0:1], EXP)



            ident = persist.tile([128, 128], F32, tag="ident")
            make_identity(nc, ident[:])
            ident_bf = persist.tile([128, 128], BF16, tag="identbf")
            nc.vector.tensor_copy(ident_bf[:], ident[:])
            ones1 = persist.tile([1, 128], BF16, tag="ones1")
            nc.vector.tensor_copy(ones1[:], ones1_f[:])
            ones4_f = persist.tile([128, HPC], F32, tag="ones4f")
            nc.vector.memset(ones4_f[:], 1.0)

            def hid_g(g, j):
                # 256-wide column group g of hiddenT chunk j (g = s//256)
                if g < 2:
                    return hidA[j][:, (g % 2) * 256 : (g % 2 + 1) * 256]
                if g < 4:
                    return hidB[j][:, (g % 2) * 256 : (g % 2 + 1) * 256]
                return hidC[j][:, (g - 4) * 256 : (g - 3) * 256]

            def hid_s(s, j):
                # 128-wide column chunk s of hiddenT chunk j
                if s < 4:
                    return hidA[j][:, s * 128 : (s + 1) * 128]
                if s < 8:
                    return hidB[j][:, (s - 4) * 128 : (s - 3) * 128]
                return hidC[j][:, (s - 8) * 128 : (s - 7) * 128]

            # persistent activations
            qT = [
                persist.tile([128, S], BF16, tag=f"qT{p}", name=f"qT{p}")
                for p in range(2)
            ]
            kT = [
                persist.tile([128, S], BF16, tag=f"kT{p}", name=f"kT{p}")
                for p in range(2)
            ]
            # v padded to 128 columns per head (cols 65-127 zero) so the ctx
            # matmul fills all 128 PSUM partitions -- the X-bar output
            # transpose then moves a fully-written [128,128] block
            v_sb = [
                persist.tile([128, HPC, 128], BF16, tag=f"v{s}", name=f"v{s}")
                for s in range(SC)
            ]
            for s in range(SC):
                nc.vector.memset(v_sb[s][:, :, HD + 1 : 128], 0.0)
            out_sb = [
                persist.tile([128, DPC], F32, tag=f"o{s}", name=f"o{s}")
                for s in range(SC)
            ]
            # staged k0-7 partial contexts for the split qbs: (qb, hh)
            pctx = {
                (qb, hh): persist.tile(
                    [128, QB], F32, tag=f"pc{qb}{hh}", name=f"pc{qb}{hh}"
                )
                for qb in (0, 1)
                for hh in (0, 1)
            }

            # PE warmup: ramp the p-state (full clock needs 3us continuous
            # busy) while the DMA chain delivers. Results are discarded.
            wps = scps.tile([128, 512], F32, tag="sc", name="warmup_ps")
            for _ in range(28):
                nc.tensor.matmul(wps[:], dummyw[:], dummy[:], start=True, stop=True)

            def qk_group(wname, dst, bias, p, g):
                # one 256-wide output group of the qT/kT projection
                ps = scps.tile([128, 256], F32, tag="sc", name="projps_t")
                for j in range(JC):
                    nc.tensor.matmul(
                        ps[:],
                        w_sb[wname][:, j, p * 128 : (p + 1) * 128],
                        hid_g(g, j),
                        start=(j == 0),
                        stop=(j == JC - 1),
                    )
                nc.vector.tensor_scalar_add(
                    dst[p][:, g * 256 : (g + 1) * 256], ps[:], bias[:, p : p + 1]
                )

            def qk_pieces(wname, dst, bias, p, g):
                # the same group as two 4-chunk pieces for smooth filling
                box = {}

                def piece_a():
                    ps = scps.tile([128, 256], F32, tag="sc", name="projps_t")
                    box["ps"] = ps
                    for j in range(4):
                        nc.tensor.matmul(
                            ps[:],
                            w_sb[wname][:, j, p * 128 : (p + 1) * 128],
                            hid_g(g, j),
                            start=(j == 0),
                            stop=False,
                        )

                def piece_b():
                    ps = box["ps"]
                    for j in range(4, JC):
                        nc.tensor.matmul(
                            ps[:],
                            w_sb[wname][:, j, p * 128 : (p + 1) * 128],
                            hid_g(g, j),
                            start=False,
                            stop=(j == JC - 1),
                        )
                    nc.vector.tensor_scalar_add(
                        dst[p][:, g * 256 : (g + 1) * 256], ps[:], bias[:, p : p + 1]
                    )

                return piece_a, piece_b

            def v_proj(s):
                ps = scps.tile([128, DPC], F32, tag="sc", name="vps_t")
                for j in range(JC):
                    nc.tensor.matmul(
                        ps[:],
                        hid_s(s, j),
                        w_sb["wv"][:, j, :],
                        start=(j == 0),
                        stop=False,
                    )
                nc.tensor.matmul(ps[:], ones1[:], bvs_sb[:], start=False, stop=True)
                ps3 = ps.rearrange("p (h c) -> p h c", h=HPC)
                nc.vector.tensor_copy(v_sb[s][:, :, 0:HD], ps3[:])
                nc.vector.tensor_copy(
                    v_sb[s][:, :, HD : HD + 1],
                    ones4_f[:].rearrange("p (h o) -> p h o", o=1),
                )

            # prologue projections, j-interleaved so each 8-matmul batch
            # tracks the hidA transpose staircase just-in-time:
            # kT[0] cols 0-255, qT[0] cols 0-511
            pro = [
                scps.tile([128, 256], F32, tag="sc", name=f"props{n}")
                for n in range(3)
            ]
            for j in range(JC):
                nc.tensor.matmul(
                    pro[0][:], w_sb["wk"][:, j, 0:128], hid_g(0, j),
                    start=(j == 0), stop=(j == JC - 1),
                )
                nc.tensor.matmul(
                    pro[1][:], w_sb["wq"][:, j, 0:128], hid_g(0, j),
                    start=(j == 0), stop=(j == JC - 1),
                )
                nc.tensor.matmul(
                    pro[2][:], w_sb["wq"][:, j, 0:128], hid_g(1, j),
                    start=(j == 0), stop=(j == JC - 1),
                )
            nc.vector.tensor_scalar_add(kT[0][:, 0:256], pro[0][:], bks_sb[:, 0:1])
            nc.vector.tensor_scalar_add(qT[0][:, 0:256], pro[1][:], bqs_sb[:, 0:1])
            nc.vector.tensor_scalar_add(qT[0][:, 256:512], pro[2][:], bqs_sb[:, 0:1])

            # Filler schedules over flat iters. dlfill runs BEFORE the
            # sc(i+2) emission (only groups sc(i+2) reads -- anything before
            # a sc inflates the PE-counter threshold its exp waits on);
            # fillers runs AFTER sc(i+2), before ctx(i). Two-piece fillers
            # land on consecutive iters.
            dlfill = {}
            fillers = {}

            def dl(it, fn):
                dlfill.setdefault(it, []).append(fn)

            def fl(it, fn):
                fillers.setdefault(it, []).append(fn)

            def kg(p, g):
                return lambda: qk_group("wk", kT, bks_sb, p, g)

            def qg(p, g):
                return lambda: qk_group("wq", qT, bqs_sb, p, g)

            def place2(pk, qw, dst, bias, p, g, it):
                a, b = qk_pieces(qw, dst, bias, p, g)
                pk.setdefault(it, []).append(a)
                pk.setdefault(it + 1, []).append(b)

            # deadline groups, wall-aligned: iters 0-3 emit ONLY hidA-
            # dependent work; the first hidB-dependent instruction is
            # dlfill[4] and the first hidC-dependent one is vp(8)/kg(0,4)
            # at iters >= 14, so the PE queue never blocks on a late
            # transpose while earlier-data work is runnable behind it.
            dl(0, kg(0, 1))
            dl(4, kg(0, 2))
            dl(4, kg(0, 3))
            dl(5, qg(0, 2))
            dl(5, qg(0, 3))
            dl(14, kg(0, 4))
            dl(16, kg(0, 5))
            dl(18, kg(0, 6))
            dl(20, kg(0, 7))
            dl(60, qg(1, 0))
            dl(61, qg(1, 1))
            # v-projections: v0-7 (hidA/hidB) at their consuming iters;
            # v8-15 (hidC) just-in-time before the k8-15 passes
            for s in range(8):
                fl(s, lambda s=s: v_proj(s))
            for s in range(8, SC):
                fl(s + 7, lambda s=s: v_proj(s))
            # cruise fillers as 2-piece pairs (~0.33us/iter when spaced two
            # apart), after their data exists and before their consumer's
            # sc emission
            place2(fillers, "wq", qT, bqs_sb, 0, 4, 25)
            place2(fillers, "wq", qT, bqs_sb, 0, 5, 27)
            for g in range(GC):
                place2(fillers, "wk", kT, bks_sb, 1, g, 29 + 4 * g)
            place2(fillers, "wq", qT, bqs_sb, 0, 6, 39)
            place2(fillers, "wq", qT, bqs_sb, 0, 7, 43)
            place2(fillers, "wq", qT, bqs_sb, 1, 2, 65)
            place2(fillers, "wq", qT, bqs_sb, 1, 3, 69)
            place2(fillers, "wq", qT, bqs_sb, 1, 4, 81)
            place2(fillers, "wq", qT, bqs_sb, 1, 5, 85)
            place2(fillers, "wq", qT, bqs_sb, 1, 6, 97)
            place2(fillers, "wq", qT, bqs_sb, 1, 7, 101)

            # sc emission schedule: lead-2 except across the hidB wall,
            # where sc(4..7) wait for the iter-4/5 deadline groups
            EMITS = {i: [i + 2] for i in range(NIT - 2)}
            EMITS[2] = []
            EMITS[3] = []
            EMITS[4] = [4, 5]
            EMITS[5] = [6, 7]
            EMITS[6] = [8]
            EMITS[7] = [9]

            sts = {}

            def emit_scores(i):
                pair, qb, k = ITERS[i]
                st = scps.tile([128, 2 * QB], F32, tag="sc", name="sc_t")
                qs = qb * QB
                # adjacent emission, opposite row groups -> the PE runs
                # these two K=64 matmuls concurrently
                nc.tensor.matmul(
                    st[:, 0:QB],
                    kT[pair][0:64, k * 128 : (k + 1) * 128],
                    qT[pair][0:64, qs : qs + QB],
                    start=True,
                    stop=True,
                )
                nc.tensor.matmul(
                    st[:, QB : 2 * QB],
                    kT[pair][64:128, k * 128 : (k + 1) * 128],
                    qT[pair][64:128, qs : qs + QB],
                    start=True,
                    stop=True,
                )
                sts[i] = st

            emit_scores(0)
            emit_scores(1)
            ctxs = {}
            for i in range(NIT):
                pair, qb, k = ITERS[i]
                h0, h1 = 2 * pair, 2 * pair + 1
                split = (pair, qb) in SPLIT_QBS
                if k == 0 or (split and k == 8):
                    ctxs[0] = ctxps.tile([128, QB], F32, tag="ctx0", name="ctx0")
                    ctxs[1] = ctxps.tile([128, QB], F32, tag="ctx1", name="ctx1")
                for fn in dlfill.get(i, ()):
                    fn()
                for j in EMITS.get(i, ()):
                    emit_scores(j)
                st = sts.pop(i)
                et = etp.tile([128, 2 * QB], BF16, tag="et", name="et_t")
                nc.scalar.activation(
                    et[:], st[:], EXP, bias=mask_sb[:, k : k + 1], scale=1.0
                )
                for fn in fillers.get(i, ()):
                    fn()
                cstart = k == 0 or (split and k == 8)
                cstop = k == SC - 1 or (split and k == 7)
                nc.tensor.matmul(
                    ctxs[0][:], v_sb[k][:, h0, :], et[:, 0:QB],
                    start=cstart, stop=cstop,
                )
                nc.tensor.matmul(
                    ctxs[1][:], v_sb[k][:, h1, :], et[:, QB : 2 * QB],
                    start=cstart, stop=cstop,
                )
                if split and k == 7:
                    # stage the k0-7 partial context to SBUF f32
                    for hh in (0, 1):
                        nc.vector.tensor_copy(pctx[(qb, hh)][:], ctxs[hh][:])
                if k == SC - 1:
                    # finalize: (merge +) copy ctx to SBUF bf16 (rows 65-127
                    # are pre-initialized padding), transpose each 128-col
                    # chunk on the now-idle X-bar (sync queue) instead of
                    # the PE, normalize, stage to out_sb, DMA out
                    ctss = {}
                    for hh in (0, 1):
                        cts = ctsp.tile([128, QB], BF16, tag="cts", name="cts_t")
                        if split:
                            nc.vector.tensor_add(
                                cts[:], ctxs[hh][:], pctx[(qb, hh)][:]
                            )
                        else:
                            nc.vector.tensor_copy(cts[:], ctxs[hh][:])
                        ctss[hh] = cts
                    for ci in range(QB // 128):
                        for hh in (0, 1):
                            h = 2 * pair + hh
                            tp = tpsb.tile([128, 128], BF16, tag="tp", name="tp_t")
                            nc.sync.dma_start_transpose(
                                out=tp[:],
                                in_=ctss[hh][:, ci * 128 : (ci + 1) * 128],
                            )
                            rc = rcp.tile([128, 1], F32, tag="rc", name="rc_t")
                            nc.vector.reciprocal(rc[:], tp[:, HD : HD + 1])
                            qc = qb * (QB // 128) + ci
                            nc.vector.tensor_scalar_mul(
                                out_sb[qc][:, h * HD : (h + 1) * HD],
                                tp[:, 0:HD],
                                rc[:],
                            )
                            if i == NIT - 1:
                                nc.sync.dma_start(
                                    out[
                                        qc * 128 : (qc + 1) * 128,
                                        h * HD : (h + 1) * HD,
                                    ],
                                    out_sb[qc][:, h * HD : (h + 1) * HD],
                                )
                    if i < NIT - 1:
                        for ci in range(QB // 128):
                            qc = qb * (QB // 128) + ci
                            nc.sync.dma_start(
                                out[
                                    qc * 128 : (qc + 1) * 128,
                                    pair * 128 : (pair + 1) * 128,
                                ],
                                out_sb[qc][:, pair * 128 : (pair + 1) * 128],
                            )

    nc.compile()
    return nc


def make_in_maps(hidden_states, attention_mask, Wq, bq, Wk, bk, Wv, bv):
    hidden_states = np.asarray(hidden_states, dtype=np.float32)
    attention_mask = np.asarray(attention_mask, dtype=np.float32)
    Wq = np.asarray(Wq, dtype=np.float32)
    bq = np.asarray(bq, dtype=np.float32)
    Wk = np.asarray(Wk, dtype=np.float32)
    bk = np.asarray(bk, dtype=np.float32)
    Wv = np.asarray(Wv, dtype=np.float32)
    bv = np.asarray(bv, dtype=np.float32)
    bf = ml_dtypes.bfloat16

    def warr(w):
        # [H, DPC] -> [128, JC, DPC]: partition-major, contiguous DMA
        return w.reshape(JC, 128, DPC).transpose(1, 0, 2)

    in_maps = []
    for c in range(NCORES):
        b = c // 4
        g = c % 4
        rows = slice(g * DPC, (g + 1) * DPC)
        wkq = np.stack(
            [warr(Wk[rows, :].T), warr((Wq[rows, :] * 0.125).T)], axis=1
        )
        consts = np.concatenate(
            [
                attention_mask[b, 0, 0, :].reshape(SC, 128).T,
                (bq[rows] * 0.125).reshape(2, 128).T,
                bk[rows].reshape(2, 128).T,
            ],
            axis=1,
        )
        in_maps.append(
            {
                "hidb": np.ascontiguousarray(hidden_states[b]).astype(bf),
                "wkq": np.ascontiguousarray(wkq).astype(bf),
                "wvd": np.ascontiguousarray(warr(Wv[rows, :].T)).astype(bf),
                "consts": np.ascontiguousarray(consts.astype(np.float32)),
                "bvs": np.ascontiguousarray(bv[rows].reshape(1, DPC)).astype(bf),
            }
        )
    return in_maps


def gather(results):
    full = np.empty((B, S, H), dtype=np.float32)
    for c in range(NCORES):
        b = c // 4
        g = c % 4
        full[b, :, g * DPC : (g + 1) * DPC] = results[c]["out"]
    return full


_NC = None


def kernel(hidden_states, attention_mask, Wq, bq, Wk, bk, Wv, bv, **run_kwargs):
    global _NC
    if _NC is None:
        _NC = build()
    in_maps = make_in_maps(hidden_states, attention_mask, Wq, bq, Wk, bk, Wv, bv)
    res = run_bass_kernel_spmd(_NC, in_maps, core_ids=list(range(NCORES)), **run_kwargs)
    out = gather(res.results)
    if run_kwargs:
        kernel.last_result = res
    return out


# revision 35
# speedup vs baseline: 1.2450x; 1.2450x over previous
"""BertSelfAttention (B=2, S=2048, H=1024, 16 heads x 64) on 8 TRN2 NeuronCores.

Sharding: data parallel on batch (4 cores per batch) x tensor parallel on
heads (4 heads per core). No cross-core comms; each core computes
out[b, :, 256*g:256*(g+1)] for its head group g.

v10 design notes (all measured on HW traces):
- Every DMA around an X-bar transpose serializes on the previous DMA's
  completion (X-bar quiescence), so startup is ONE serial chain on the sync
  queue ordered by consumption: consts | wk+wq | bvs | T(s0-511) | wv |
  T(s512-1023) | T(s1024-2047). Weights are host-pre-arranged so their DMA
  is contiguous per partition.
- Tile deps compile to monotonic per-engine counters: exp(i) waits the PE
  counter at its sc(i) emission, so anything emitted before a sc inflates
  the exp critical path. Deadline projections (dlfill) are the only work
  emitted before sc(i+2); all other fillers go after.
- The PE p-state reaches full clock only after ~3us of continuous busy; a
  discarded warmup matmul chain covers the DMA-chain wait.
- Iteration order interleaves qb0/qb1 first halves so the ACT exp stream
  has work while the PE grinds v-projections, and the k>=8 data (last
  transposes) arrives before anything needs it. qb0/qb1 k0-7 context is
  staged to SBUF f32 and merged in the k8-15 pass.
- Score PSUM pool has 3 slots (lead-2 emission); projection PSUM shares the
  same slots via tag-sharing; output transposes reuse the two ctx banks
  (plus idle score slots on the final drain). 8 PSUM banks exactly.

Per-core pipeline:
  A) hiddenT via 24 serial X-bar transposes ([512,128]x16 + [1024,128]x8)
  B) kT/qT [128(d of pair), 2048(s)] bf16 (1/8 scale + bias folded) via
     256-col groups; V [128(s), 4heads, 65] bf16 with ones column
  C) per (pair, qb, k): scoresT h0|h1 packed -> one exp [128,1024] (mask
     bias) -> bf16 et, ctxT[65, 512] += v_ext.T @ et per head
  D) PE-transpose ctxT (bf16) -> [q, 65], DVE reciprocal + scale, DMA out
     per (pair, qb) half-block during the stream
"""

import ml_dtypes
import numpy as np

import concourse.bass as bass
import concourse.tile as tile
from concourse import bacc, mybir
from concourse.bass_utils import run_bass_kernel_spmd
from concourse.masks import make_identity

F32 = mybir.dt.float32
BF16 = mybir.dt.bfloat16
EXP = mybir.ActivationFunctionType.Exp

B, S, H = 2, 2048, 1024
NH, HD = 16, 64
NCORES = 8
HPC = 4  # heads per core
DPC = HPC * HD  # 256 output dims per core
SC = S // 128  # 16 s/k chunks
JC = H // 128  # 8 contraction chunks
QB = 512  # q block in attention inner loop
NQB = S // QB  # 4
GC = 8  # 256-col projection groups per (w, pair)

# flat iteration order: qb0/qb1 first halves interleaved into the
# v-projection grind, second halves after the late transposes land,
# then qb2/qb3 and pair 1 straight
ITERS = (
    [(0, 0, k) for k in range(8)]
    + [(0, 1, k) for k in range(8)]
    + [(0, 0, k) for k in range(8, SC)]
    + [(0, 1, k) for k in range(8, SC)]
    + [(0, 2, k) for k in range(SC)]
    + [(0, 3, k) for k in range(SC)]
    + [(1, qb, k) for qb in range(NQB) for k in range(SC)]
)
NIT = len(ITERS)  # 128
SPLIT_QBS = {(0, 0), (0, 1)}  # qbs whose k0-7 context is staged and merged


def build():
    nc = bacc.Bacc(
        "TRN2",
        target_bir_lowering=False,
        debug=False,
        enable_asserts=False,
        num_devices=NCORES,
    )
    hidb = nc.dram_tensor("hidb", [S, H], BF16, kind="ExternalInput").ap()
    wkq = nc.dram_tensor("wkq", [128, 2, JC, DPC], BF16, kind="ExternalInput").ap()
    wvd = nc.dram_tensor("wvd", [128, JC, DPC], BF16, kind="ExternalInput").ap()
    # mask | bqs | bks packed into one [128, SC+4] f32 tensor
    consts = nc.dram_tensor("consts", [128, SC + 4], F32, kind="ExternalInput").ap()
    bvs = nc.dram_tensor("bvs", [1, DPC], BF16, kind="ExternalInput").ap()
    out = nc.dram_tensor("out", [S, DPC], F32, kind="ExternalOutput").ap()

    with tile.TileContext(nc) as tc:
        with (
            tc.tile_pool(name="persist", bufs=1) as persist,
            tc.tile_pool(name="etp", bufs=8) as etp,
            tc.tile_pool(name="ctsp", bufs=2) as ctsp,
            tc.tile_pool(name="tpsb", bufs=4) as tpsb,
            tc.tile_pool(name="rcp", bufs=4) as rcp,
            tc.tile_pool(name="scps", bufs=3, space="PSUM") as scps,
            tc.tile_pool(name="ctxps", bufs=1, space="PSUM") as ctxps,
        ):
            # dummy operands for the PE warmup chain
            dummyw = persist.tile([128, 128], BF16, tag="dummyw")
            nc.vector.memset(dummyw[:], 0.0)
            dummy = persist.tile([128, 512], BF16, tag="dummy")
            nc.vector.memset(dummy[:], 0.0)

            # startup DMA chain on the sync queue (see module docstring)
            consts_sb = persist.tile([128, SC + 4], F32, tag="consts")
            nc.sync.dma_start(consts_sb[:], consts)
            mask_sb = consts_sb[:, 0:SC]
            bqs_sb = consts_sb[:, SC : SC + 2]
            bks_sb = consts_sb[:, SC + 2 : SC + 4]
            wkq_t = persist.tile([128, 2, JC, DPC], BF16, tag="wkq", name="w_kq")
            nc.sync.dma_start(wkq_t[:], wkq)
            w_sb = {"wk": wkq_t[:, 0], "wq": wkq_t[:, 1]}
            bvs_sb = persist.tile([1, DPC], BF16, tag="bvs")
            nc.sync.dma_start(bvs_sb[:], bvs)

            hidA = [
                persist.tile([128, 512], BF16, tag=f"hA{j}", name=f"hA{j}")
                for j in range(JC)
            ]
            hidB = [
                persist.tile([128, 512], BF16, tag=f"hB{j}", name=f"hB{j}")
                for j in range(JC)
            ]
            hidC = [
                persist.tile([128, 1024], BF16, tag=f"hC{j}", name=f"hC{j}")
                for j in range(JC)
            ]
            for j in range(JC):
                nc.sync.dma_start_transpose(
                    out=hidA[j][:], in_=hidb[0:512, j * 128 : (j + 1) * 128]
                )
            wv_t = persist.tile([128, JC, DPC], BF16, tag="wv", name="w_wv")
            nc.sync.dma_start(wv_t[:], wvd)
            w_sb["wv"] = wv_t
            for j in range(JC):
                nc.sync.dma_start_transpose(
                    out=hidB[j][:], in_=hidb[512:1024, j * 128 : (j + 1) * 128]
                )
            for j in range(JC):
                nc.sync.dma_start_transpose(
                    out=hidC[j][:], in_=hidb[1024:S, j * 128 : (j + 1) * 128]
                )

            ones1_f = persist.tile([1, 128], F32, tag="ones1f")
            nc.vector.memset(ones1_f[:], 1.0)
            # warm the ACT exp table during startup
            warm = persist.tile([1, 1], F32, tag="warm")
            nc.scalar.activation(warm[:], ones1_f[:, 0:1], EXP)



            ident = persist.tile([128, 128], F32, tag="ident")
            make_identity(nc, ident[:])
            ident_bf = persist.tile([128, 128], BF16, tag="identbf")
            nc.vector.tensor_copy(ident_bf[:], ident[:])
            ones1 = persist.tile([1, 128], BF16, tag="ones1")
            nc.vector.tensor_copy(ones1[:], ones1_f[:])
            ones4_f = persist.tile([128, HPC], F32, tag="ones4f")
            nc.vector.memset(ones4_f[:], 1.0)

            def hid_g(g, j):
                # 256-wide column group g of hiddenT chunk j (g = s//256)
                if g < 2:
                    return hidA[j][:, (g % 2) * 256 : (g % 2 + 1) * 256]
                if g < 4:
                    return hidB[j][:, (g % 2) * 256 : (g % 2 + 1) * 256]
                return hidC[j][:, (g - 4) * 256 : (g - 3) * 256]

            def hid_s(s, j):
                # 128-wide column chunk s of hiddenT chunk j
                if s < 4:
                    return hidA[j][:, s * 128 : (s + 1) * 128]
                if s < 8:
                    return hidB[j][:, (s - 4) * 128 : (s - 3) * 128]
                return hidC[j][:, (s - 8) * 128 : (s - 7) * 128]

            # persistent activations
            qT = [
                persist.tile([128, S], BF16, tag=f"qT{p}", name=f"qT{p}")
                for p in range(2)
            ]
            kT = [
                persist.tile([128, S], BF16, tag=f"kT{p}", name=f"kT{p}")
                for p in range(2)
            ]
            # v padded to 128 columns per head (cols 65-127 zero) so the ctx
            # matmul fills all 128 PSUM partitions -- the X-bar output
            # transpose then moves a fully-written [128,128] block
            v_sb = [
                persist.tile([128, HPC, 128], BF16, tag=f"v{s}", name=f"v{s}")
                for s in range(SC)
            ]
            out_sb = [
                persist.tile([128, DPC], F32, tag=f"o{s}", name=f"o{s}")
                for s in range(SC)
            ]
            # staged k0-7 partial contexts for the split qbs: (qb, hh)
            pctx = {
                (qb, hh): persist.tile(
                    [128, QB], F32, tag=f"pc{qb}{hh}", name=f"pc{qb}{hh}"
                )
                for qb in (0, 1)
                for hh in (0, 1)
            }

            # PE warmup: ramp the p-state (full clock needs 3us continuous
            # busy) while the DMA chain delivers. Results are discarded.
            wps = scps.tile([128, 512], F32, tag="sc", name="warmup_ps")
            for _ in range(28):
                nc.tensor.matmul(wps[:], dummyw[:], dummy[:], start=True, stop=True)

            def qk_group(wname, dst, bias, p, g):
                # one 256-wide output group of the qT/kT projection
                ps = scps.tile([128, 256], F32, tag="sc", name="projps_t")
                for j in range(JC):
                    nc.tensor.matmul(
                        ps[:],
                        w_sb[wname][:, j, p * 128 : (p + 1) * 128],
                        hid_g(g, j),
                        start=(j == 0),
                        stop=(j == JC - 1),
                    )
                nc.vector.tensor_scalar_add(
                    dst[p][:, g * 256 : (g + 1) * 256], ps[:], bias[:, p : p + 1]
                )

            def qk_pieces(wname, dst, bias, p, g):
                # the same group as two 4-chunk pieces for smooth filling
                box = {}

                def piece_a():
                    ps = scps.tile([128, 256], F32, tag="sc", name="projps_t")
                    box["ps"] = ps
                    for j in range(4):
                        nc.tensor.matmul(
                            ps[:],
                            w_sb[wname][:, j, p * 128 : (p + 1) * 128],
                            hid_g(g, j),
                            start=(j == 0),
                            stop=False,
                        )

                def piece_b():
                    ps = box["ps"]
                    for j in range(4, JC):
                        nc.tensor.matmul(
                            ps[:],
                            w_sb[wname][:, j, p * 128 : (p + 1) * 128],
                            hid_g(g, j),
                            start=False,
                            stop=(j == JC - 1),
                        )
                    nc.vector.tensor_scalar_add(
                        dst[p][:, g * 256 : (g + 1) * 256], ps[:], bias[:, p : p + 1]
                    )

                return piece_a, piece_b

            def v_proj(s):
                ps = scps.tile([128, DPC], F32, tag="sc", name="vps_t")
                for j in range(JC):
                    nc.tensor.matmul(
                        ps[:],
                        hid_s(s, j),
                        w_sb["wv"][:, j, :],
                        start=(j == 0),
                        stop=False,
                    )
                nc.tensor.matmul(ps[:], ones1[:], bvs_sb[:], start=False, stop=True)
                ps3 = ps.rearrange("p (h c) -> p h c", h=HPC)
                nc.vector.tensor_copy(v_sb[s][:, :, 0:HD], ps3[:])
                nc.vector.tensor_copy(
                    v_sb[s][:, :, HD : HD + 1],
                    ones4_f[:].rearrange("p (h o) -> p h o", o=1),
                )

            # prologue projections, j-interleaved so each 8-matmul batch
            # tracks the hidA transpose staircase just-in-time:
            # kT[0] cols 0-255, qT[0] cols 0-511
            pro = [
                scps.tile([128, 256], F32, tag="sc", name=f"props{n}")
                for n in range(3)
            ]
            for j in range(JC):
                nc.tensor.matmul(
                    pro[0][:], w_sb["wk"][:, j, 0:128], hid_g(0, j),
                    start=(j == 0), stop=(j == JC - 1),
                )
                nc.tensor.matmul(
                    pro[1][:], w_sb["wq"][:, j, 0:128], hid_g(0, j),
                    start=(j == 0), stop=(j == JC - 1),
                )
                nc.tensor.matmul(
                    pro[2][:], w_sb["wq"][:, j, 0:128], hid_g(1, j),
                    start=(j == 0), stop=(j == JC - 1),
                )
            nc.vector.tensor_scalar_add(kT[0][:, 0:256], pro[0][:], bks_sb[:, 0:1])
            nc.vector.tensor_scalar_add(qT[0][:, 0:256], pro[1][:], bqs_sb[:, 0:1])
            nc.vector.tensor_scalar_add(qT[0][:, 256:512], pro[2][:], bqs_sb[:, 0:1])
            # zero the v padding columns (after the prologue bias-adds so
            # they don't delay the first scores on the DVE queue)
            for s in range(SC):
                nc.vector.memset(v_sb[s][:, :, HD + 1 : 128], 0.0)

            # Filler schedules over flat iters. dlfill runs BEFORE the
            # sc(i+2) emission (only groups sc(i+2) reads -- anything before
            # a sc inflates the PE-counter threshold its exp waits on);
            # fillers runs AFTER sc(i+2), before ctx(i). Two-piece fillers
            # land on consecutive iters.
            dlfill = {}
            fillers = {}

            def dl(it, fn):
                dlfill.setdefault(it, []).append(fn)

            def fl(it, fn):
                fillers.setdefault(it, []).append(fn)

            def kg(p, g):
                return lambda: qk_group("wk", kT, bks_sb, p, g)

            def qg(p, g):
                return lambda: qk_group("wq", qT, bqs_sb, p, g)

            def place2(pk, qw, dst, bias, p, g, it):
                a, b = qk_pieces(qw, dst, bias, p, g)
                pk.setdefault(it, []).append(a)
                pk.setdefault(it + 1, []).append(b)

            # deadline groups, wall-aligned: iters 0-3 emit ONLY hidA-
            # dependent work; the first hidB-dependent instruction is
            # dlfill[4] and the first hidC-dependent one is vp(8)/kg(0,4)
            # at iters >= 14, so the PE queue never blocks on a late
            # transpose while earlier-data work is runnable behind it.
            dl(0, kg(0, 1))
            dl(4, kg(0, 2))
            dl(4, kg(0, 3))
            dl(5, qg(0, 2))
            dl(5, qg(0, 3))
            dl(14, kg(0, 4))
            dl(16, kg(0, 5))
            dl(18, kg(0, 6))
            dl(20, kg(0, 7))
            dl(60, qg(1, 0))
            dl(61, qg(1, 1))
            # v-projections: v0-7 (hidA/hidB) at their consuming iters;
            # v8-15 (hidC) just-in-time before the k8-15 passes
            for s in range(8):
                fl(s, lambda s=s: v_proj(s))
            for s in range(8, SC):
                fl(s + 7, lambda s=s: v_proj(s))
            # cruise fillers as 2-piece pairs (~0.33us/iter when spaced two
            # apart), after their data exists and before their consumer's
            # sc emission
            place2(fillers, "wq", qT, bqs_sb, 0, 4, 25)
            place2(fillers, "wq", qT, bqs_sb, 0, 5, 27)
            for g in range(GC):
                place2(fillers, "wk", kT, bks_sb, 1, g, 29 + 4 * g)
            place2(fillers, "wq", qT, bqs_sb, 0, 6, 39)
            place2(fillers, "wq", qT, bqs_sb, 0, 7, 43)
            place2(fillers, "wq", qT, bqs_sb, 1, 2, 65)
            place2(fillers, "wq", qT, bqs_sb, 1, 3, 69)
            place2(fillers, "wq", qT, bqs_sb, 1, 4, 81)
            place2(fillers, "wq", qT, bqs_sb, 1, 5, 85)
            place2(fillers, "wq", qT, bqs_sb, 1, 6, 97)
            place2(fillers, "wq", qT, bqs_sb, 1, 7, 101)

            # sc emission schedule: lead-2 except across the hidB wall,
            # where sc(4..7) wait for the iter-4/5 deadline groups
            EMITS = {i: [i + 2] for i in range(NIT - 2)}
            EMITS[2] = []
            EMITS[3] = []
            EMITS[4] = [4, 5]
            EMITS[5] = [6, 7]
            EMITS[6] = [8]
            EMITS[7] = [9]

            sts = {}

            def emit_scores(i):
                pair, qb, k = ITERS[i]
                st = scps.tile([128, 2 * QB], F32, tag="sc", name="sc_t")
                qs = qb * QB
                # adjacent emission, opposite row groups -> the PE runs
                # these two K=64 matmuls concurrently
                nc.tensor.matmul(
                    st[:, 0:QB],
                    kT[pair][0:64, k * 128 : (k + 1) * 128],
                    qT[pair][0:64, qs : qs + QB],
                    start=True,
                    stop=True,
                )
                nc.tensor.matmul(
                    st[:, QB : 2 * QB],
                    kT[pair][64:128, k * 128 : (k + 1) * 128],
                    qT[pair][64:128, qs : qs + QB],
                    start=True,
                    stop=True,
                )
                sts[i] = st

            emit_scores(0)
            emit_scores(1)
            ctxs = {}
            for i in range(NIT):
                pair, qb, k = ITERS[i]
                h0, h1 = 2 * pair, 2 * pair + 1
                split = (pair, qb) in SPLIT_QBS
                if k == 0 or (split and k == 8):
                    ctxs[0] = ctxps.tile([128, QB], F32, tag="ctx0", name="ctx0")
                    ctxs[1] = ctxps.tile([128, QB], F32, tag="ctx1", name="ctx1")
                for fn in dlfill.get(i, ()):
                    fn()
                for j in EMITS.get(i, ()):
                    emit_scores(j)
                st = sts.pop(i)
                et = etp.tile([128, 2 * QB], BF16, tag="et", name="et_t")
                nc.scalar.activation(
                    et[:], st[:], EXP, bias=mask_sb[:, k : k + 1], scale=1.0
                )
                for fn in fillers.get(i, ()):
                    fn()
                cstart = k == 0 or (split and k == 8)
                cstop = k == SC - 1 or (split and k == 7)
                nc.tensor.matmul(
                    ctxs[0][:], v_sb[k][:, h0, :], et[:, 0:QB],
                    start=cstart, stop=cstop,
                )
                nc.tensor.matmul(
                    ctxs[1][:], v_sb[k][:, h1, :], et[:, QB : 2 * QB],
                    start=cstart, stop=cstop,
                )
                if split and k == 7:
                    # stage the k0-7 partial context to SBUF f32
                    for hh in (0, 1):
                        nc.vector.tensor_copy(pctx[(qb, hh)][:], ctxs[hh][:])
                if k == SC - 1:
                    # finalize: (merge +) copy ctx to SBUF bf16 (rows 65-127
                    # are pre-initialized padding), transpose each 128-col
                    # chunk on the now-idle X-bar (sync queue) instead of
                    # the PE, normalize, stage to out_sb, DMA out
                    ctss = {}
                    for hh in (0, 1):
                        cts = ctsp.tile([128, QB], BF16, tag="cts", name="cts_t")
                        if split:
                            nc.vector.tensor_add(
                                cts[:], ctxs[hh][:], pctx[(qb, hh)][:]
                            )
                        else:
                            nc.vector.tensor_copy(cts[:], ctxs[hh][:])
                        ctss[hh] = cts
                    if i == NIT - 1:
                        slots = [
                            (ctxps, "ctx0"),
                            (ctxps, "ctx1"),
                            (scps, "sc"),
                            (scps, "sc"),
                        ]
                    else:
                        slots = [(ctxps, "ctx0"), (ctxps, "ctx1")]
                    nt = 0
                    for ci in range(QB // 128):
                        for hh in (0, 1):
                            h = 2 * pair + hh
                            pool, tag = slots[nt % len(slots)]
                            tp = pool.tile([128, 128], BF16, tag=tag, name="tp_t")
                            nt += 1
                            nc.tensor.transpose(
                                tp[:],
                                ctss[hh][:, ci * 128 : (ci + 1) * 128],
                                ident_bf[:],
                            )
                            rc = rcp.tile([128, 1], F32, tag="rc", name="rc_t")
                            nc.vector.reciprocal(rc[:], tp[:, HD : HD + 1])
                            qc = qb * (QB // 128) + ci
                            nc.vector.tensor_scalar_mul(
                                out_sb[qc][:, h * HD : (h + 1) * HD],
                                tp[:, 0:HD],
                                rc[:],
                            )
                            if i == NIT - 1:
                                nc.sync.dma_start(
                                    out[
                                        qc * 128 : (qc + 1) * 128,
                                        h * HD : (h + 1) * HD,
                                    ],
                                    out_sb[qc][:, h * HD : (h + 1) * HD],
                                )
                    if i < NIT - 1:
                        for ci in range(QB // 128):
                            qc = qb * (QB // 128) + ci
                            nc.sync.dma_start(
                                out[
                                    qc * 128 : (qc + 1) * 128,
                                    pair * 128 : (pair + 1) * 128,
                                ],
                                out_sb[qc][:, pair * 128 : (pair + 1) * 128],
                            )

    nc.compile()
    return nc


def make_in_maps(hidden_states, attention_mask, Wq, bq, Wk, bk, Wv, bv):
    hidden_states = np.asarray(hidden_states, dtype=np.float32)
    attention_mask = np.asarray(attention_mask, dtype=np.float32)
    Wq = np.asarray(Wq, dtype=np.float32)
    bq = np.asarray(bq, dtype=np.float32)
    Wk = np.asarray(Wk, dtype=np.float32)
    bk = np.asarray(bk, dtype=np.float32)
    Wv = np.asarray(Wv, dtype=np.float32)
    bv = np.asarray(bv, dtype=np.float32)
    bf = ml_dtypes.bfloat16

    def warr(w):
        # [H, DPC] -> [128, JC, DPC]: partition-major, contiguous DMA
        return w.reshape(JC, 128, DPC).transpose(1, 0, 2)

    in_maps = []
    for c in range(NCORES):
        b = c // 4
        g = c % 4
        rows = slice(g * DPC, (g + 1) * DPC)
        wkq = np.stack(
            [warr(Wk[rows, :].T), warr((Wq[rows, :] * 0.125).T)], axis=1
        )
        consts = np.concatenate(
            [
                attention_mask[b, 0, 0, :].reshape(SC, 128).T,
                (bq[rows] * 0.125).reshape(2, 128).T,
                bk[rows].reshape(2, 128).T,
            ],
            axis=1,
        )
        in_maps.append(
            {
                "hidb": np.ascontiguousarray(hidden_states[b]).astype(bf),
                "wkq": np.ascontiguousarray(wkq).astype(bf),
                "wvd": np.ascontiguousarray(warr(Wv[rows, :].T)).astype(bf),
                "consts": np.ascontiguousarray(consts.astype(np.float32)),
                "bvs": np.ascontiguousarray(bv[rows].reshape(1, DPC)).astype(bf),
            }
        )
    return in_maps


def gather(results):
    full = np.empty((B, S, H), dtype=np.float32)
    for c in range(NCORES):
        b = c // 4
        g = c % 4
        full[b, :, g * DPC : (g + 1) * DPC] = results[c]["out"]
    return full


_NC = None


def kernel(hidden_states, attention_mask, Wq, bq, Wk, bk, Wv, bv, **run_kwargs):
    global _NC
    if _NC is None:
        _NC = build()
    in_maps = make_in_maps(hidden_states, attention_mask, Wq, bq, Wk, bk, Wv, bv)
    res = run_bass_kernel_spmd(_NC, in_maps, core_ids=list(range(NCORES)), **run_kwargs)
    out = gather(res.results)
    if run_kwargs:
        kernel.last_result = res
    return out


# revision 42
# speedup vs baseline: 1.2510x; 1.0049x over previous
"""BertSelfAttention (B=2, S=2048, H=1024, 16 heads x 64) on 8 TRN2 NeuronCores.

Sharding: data parallel on batch (4 cores per batch) x tensor parallel on
heads (4 heads per core). No cross-core comms; each core computes
out[b, :, 256*g:256*(g+1)] for its head group g.

v10 design notes (all measured on HW traces):
- Every DMA around an X-bar transpose serializes on the previous DMA's
  completion (X-bar quiescence), so startup is ONE serial chain on the sync
  queue ordered by consumption: consts | wk+wq | bvs | T(s0-511) | wv |
  T(s512-1023) | T(s1024-2047). Weights are host-pre-arranged so their DMA
  is contiguous per partition.
- Tile deps compile to monotonic per-engine counters: exp(i) waits the PE
  counter at its sc(i) emission, so anything emitted before a sc inflates
  the exp critical path. Deadline projections (dlfill) are the only work
  emitted before sc(i+2); all other fillers go after.
- The PE p-state reaches full clock only after ~3us of continuous busy; a
  discarded warmup matmul chain covers the DMA-chain wait.
- Iteration order interleaves qb0/qb1 first halves so the ACT exp stream
  has work while the PE grinds v-projections, and the k>=8 data (last
  transposes) arrives before anything needs it. qb0/qb1 k0-7 context is
  staged to SBUF f32 and merged in the k8-15 pass.
- Score PSUM pool has 3 slots (lead-2 emission); projection PSUM shares the
  same slots via tag-sharing; output transposes reuse the two ctx banks
  (plus idle score slots on the final drain). 8 PSUM banks exactly.

Per-core pipeline:
  A) hiddenT via 24 serial X-bar transposes ([512,128]x16 + [1024,128]x8)
  B) kT/qT [128(d of pair), 2048(s)] bf16 (1/8 scale + bias folded) via
     256-col groups; V [128(s), 4heads, 65] bf16 with ones column
  C) per (pair, qb, k): scoresT h0|h1 packed -> one exp [128,1024] (mask
     bias) -> bf16 et, ctxT[65, 512] += v_ext.T @ et per head
  D) PE-transpose ctxT (bf16) -> [q, 65], DVE reciprocal + scale, DMA out
     per (pair, qb) half-block during the stream
"""

import ml_dtypes
import numpy as np

import concourse.bass as bass
import concourse.tile as tile
from concourse import bacc, mybir
from concourse.bass_utils import run_bass_kernel_spmd
from concourse.masks import make_identity

F32 = mybir.dt.float32
BF16 = mybir.dt.bfloat16
EXP = mybir.ActivationFunctionType.Exp

B, S, H = 2, 2048, 1024
NH, HD = 16, 64
NCORES = 8
HPC = 4  # heads per core
DPC = HPC * HD  # 256 output dims per core
SC = S // 128  # 16 s/k chunks
JC = H // 128  # 8 contraction chunks
QB = 512  # q block in attention inner loop
NQB = S // QB  # 4
GC = 8  # 256-col projection groups per (w, pair)

# flat iteration order: qb0/qb1 first halves interleaved into the
# v-projection grind, second halves after the late transposes land,
# then qb2/qb3 and pair 1 straight
ITERS = (
    [(0, 0, k) for k in range(8)]
    + [(0, 1, k) for k in range(8)]
    + [(0, 0, k) for k in range(8, SC)]
    + [(0, 1, k) for k in range(8, SC)]
    + [(0, 2, k) for k in range(SC)]
    + [(0, 3, k) for k in range(SC)]
    + [(1, qb, k) for qb in range(NQB) for k in range(SC)]
)
NIT = len(ITERS)  # 128
SPLIT_QBS = {(0, 0), (0, 1)}  # qbs whose k0-7 context is staged and merged


def build():
    nc = bacc.Bacc(
        "TRN2",
        target_bir_lowering=False,
        debug=False,
        enable_asserts=False,
        num_devices=NCORES,
    )
    hidb = nc.dram_tensor("hidb", [S, H], BF16, kind="ExternalInput").ap()
    wkqv = nc.dram_tensor("wkqv", [128, 3, JC, DPC], BF16, kind="ExternalInput").ap()
    # mask | bqs | bks packed into one [128, SC+4] f32 tensor
    consts = nc.dram_tensor("consts", [128, SC + 4], F32, kind="ExternalInput").ap()
    bvs = nc.dram_tensor("bvs", [1, DPC], BF16, kind="ExternalInput").ap()
    out = nc.dram_tensor("out", [S, DPC], F32, kind="ExternalOutput").ap()

    with tile.TileContext(nc) as tc:
        with (
            tc.tile_pool(name="persist", bufs=1) as persist,
            tc.tile_pool(name="etp", bufs=8) as etp,
            tc.tile_pool(name="ctsp", bufs=2) as ctsp,
            tc.tile_pool(name="tpsb", bufs=4) as tpsb,
            tc.tile_pool(name="rcp", bufs=4) as rcp,
            tc.tile_pool(name="scps", bufs=3, space="PSUM") as scps,
            tc.tile_pool(name="ctxps", bufs=1, space="PSUM") as ctxps,
        ):
            # dummy operands for the PE warmup chain
            dummyw = persist.tile([128, 128], BF16, tag="dummyw")
            nc.vector.memset(dummyw[:], 0.0)
            dummy = persist.tile([128, 512], BF16, tag="dummy")
            nc.vector.memset(dummy[:], 0.0)

            # startup DMA chain on the sync queue (see module docstring)
            consts_sb = persist.tile([128, SC + 4], F32, tag="consts")
            nc.sync.dma_start(consts_sb[:], consts)
            mask_sb = consts_sb[:, 0:SC]
            bqs_sb = consts_sb[:, SC : SC + 2]
            bks_sb = consts_sb[:, SC + 2 : SC + 4]
            wt = persist.tile([128, 3, JC, DPC], BF16, tag="wkqv", name="w_kqv")
            nc.sync.dma_start(wt[:], wkqv)
            w_sb = {"wk": wt[:, 0], "wq": wt[:, 1], "wv": wt[:, 2]}
            bvs_sb = persist.tile([1, DPC], BF16, tag="bvs")
            nc.sync.dma_start(bvs_sb[:], bvs)

            hidA = [
                persist.tile([128, 512], BF16, tag=f"hA{j}", name=f"hA{j}")
                for j in range(JC)
            ]
            hidB = [
                persist.tile([128, 512], BF16, tag=f"hB{j}", name=f"hB{j}")
                for j in range(JC)
            ]
            hidC = [
                persist.tile([128, 1024], BF16, tag=f"hC{j}", name=f"hC{j}")
                for j in range(JC)
            ]
            for j in range(JC):
                nc.sync.dma_start_transpose(
                    out=hidA[j][:], in_=hidb[0:512, j * 128 : (j + 1) * 128]
                )
            for j in range(JC):
                nc.sync.dma_start_transpose(
                    out=hidB[j][:], in_=hidb[512:1024, j * 128 : (j + 1) * 128]
                )
            for j in range(JC):
                nc.sync.dma_start_transpose(
                    out=hidC[j][:], in_=hidb[1024:S, j * 128 : (j + 1) * 128]
                )

            ones1_f = persist.tile([1, 128], F32, tag="ones1f")
            nc.vector.memset(ones1_f[:], 1.0)
            # warm the ACT exp table during startup
            warm = persist.tile([1, 1], F32, tag="warm")
            nc.scalar.activation(warm[:], ones1_f[:, 0:1], EXP)



            ident = persist.tile([128, 128], F32, tag="ident")
            make_identity(nc, ident[:])
            ident_bf = persist.tile([128, 128], BF16, tag="identbf")
            nc.vector.tensor_copy(ident_bf[:], ident[:])
            ones1 = persist.tile([1, 128], BF16, tag="ones1")
            nc.vector.tensor_copy(ones1[:], ones1_f[:])
            ones4_f = persist.tile([128, HPC], F32, tag="ones4f")
            nc.vector.memset(ones4_f[:], 1.0)

            def hid_g(g, j):
                # 256-wide column group g of hiddenT chunk j (g = s//256)
                if g < 2:
                    return hidA[j][:, (g % 2) * 256 : (g % 2 + 1) * 256]
                if g < 4:
                    return hidB[j][:, (g % 2) * 256 : (g % 2 + 1) * 256]
                return hidC[j][:, (g - 4) * 256 : (g - 3) * 256]

            def hid_s(s, j):
                # 128-wide column chunk s of hiddenT chunk j
                if s < 4:
                    return hidA[j][:, s * 128 : (s + 1) * 128]
                if s < 8:
                    return hidB[j][:, (s - 4) * 128 : (s - 3) * 128]
                return hidC[j][:, (s - 8) * 128 : (s - 7) * 128]

            # persistent activations
            qT = [
                persist.tile([128, S], BF16, tag=f"qT{p}", name=f"qT{p}")
                for p in range(2)
            ]
            kT = [
                persist.tile([128, S], BF16, tag=f"kT{p}", name=f"kT{p}")
                for p in range(2)
            ]
            # v padded to 128 columns per head (cols 65-127 zero) so the ctx
            # matmul fills all 128 PSUM partitions -- the X-bar output
            # transpose then moves a fully-written [128,128] block
            v_sb = [
                persist.tile([128, HPC, 128], BF16, tag=f"v{s}", name=f"v{s}")
                for s in range(SC)
            ]
            out_sb = [
                persist.tile([128, DPC], F32, tag=f"o{s}", name=f"o{s}")
                for s in range(SC)
            ]
            # staged k0-7 partial contexts for the split qbs: (qb, hh)
            pctx = {
                (qb, hh): persist.tile(
                    [128, QB], F32, tag=f"pc{qb}{hh}", name=f"pc{qb}{hh}"
                )
                for qb in (0, 1)
                for hh in (0, 1)
            }

            # PE warmup: ramp the p-state (full clock needs 3us continuous
            # busy) while the DMA chain delivers. Results are discarded.
            wps = scps.tile([128, 512], F32, tag="sc", name="warmup_ps")
            for _ in range(28):
                nc.tensor.matmul(wps[:], dummyw[:], dummy[:], start=True, stop=True)

            def qk_group(wname, dst, bias, p, g):
                # one 256-wide output group of the qT/kT projection
                ps = scps.tile([128, 256], F32, tag="sc", name="projps_t")
                for j in range(JC):
                    nc.tensor.matmul(
                        ps[:],
                        w_sb[wname][:, j, p * 128 : (p + 1) * 128],
                        hid_g(g, j),
                        start=(j == 0),
                        stop=(j == JC - 1),
                    )
                nc.vector.tensor_scalar_add(
                    dst[p][:, g * 256 : (g + 1) * 256], ps[:], bias[:, p : p + 1]
                )

            def qk_pieces(wname, dst, bias, p, g):
                # the same group as two 4-chunk pieces for smooth filling
                box = {}

                def piece_a():
                    ps = scps.tile([128, 256], F32, tag="sc", name="projps_t")
                    box["ps"] = ps
                    for j in range(4):
                        nc.tensor.matmul(
                            ps[:],
                            w_sb[wname][:, j, p * 128 : (p + 1) * 128],
                            hid_g(g, j),
                            start=(j == 0),
                            stop=False,
                        )

                def piece_b():
                    ps = box["ps"]
                    for j in range(4, JC):
                        nc.tensor.matmul(
                            ps[:],
                            w_sb[wname][:, j, p * 128 : (p + 1) * 128],
                            hid_g(g, j),
                            start=False,
                            stop=(j == JC - 1),
                        )
                    nc.vector.tensor_scalar_add(
                        dst[p][:, g * 256 : (g + 1) * 256], ps[:], bias[:, p : p + 1]
                    )

                return piece_a, piece_b

            def v_proj(s):
                ps = scps.tile([128, DPC], F32, tag="sc", name="vps_t")
                for j in range(JC):
                    nc.tensor.matmul(
                        ps[:],
                        hid_s(s, j),
                        w_sb["wv"][:, j, :],
                        start=(j == 0),
                        stop=False,
                    )
                nc.tensor.matmul(ps[:], ones1[:], bvs_sb[:], start=False, stop=True)
                ps3 = ps.rearrange("p (h c) -> p h c", h=HPC)
                nc.vector.tensor_copy(v_sb[s][:, :, 0:HD], ps3[:])
                nc.vector.tensor_copy(
                    v_sb[s][:, :, HD : HD + 1],
                    ones4_f[:].rearrange("p (h o) -> p h o", o=1),
                )

            # prologue projections, j-interleaved so each 8-matmul batch
            # tracks the hidA transpose staircase just-in-time:
            # kT[0] cols 0-255, qT[0] cols 0-511
            pro = [
                scps.tile([128, 256], F32, tag="sc", name=f"props{n}")
                for n in range(3)
            ]
            for j in range(JC):
                nc.tensor.matmul(
                    pro[0][:], w_sb["wk"][:, j, 0:128], hid_g(0, j),
                    start=(j == 0), stop=(j == JC - 1),
                )
                nc.tensor.matmul(
                    pro[1][:], w_sb["wq"][:, j, 0:128], hid_g(0, j),
                    start=(j == 0), stop=(j == JC - 1),
                )
                nc.tensor.matmul(
                    pro[2][:], w_sb["wq"][:, j, 0:128], hid_g(1, j),
                    start=(j == 0), stop=(j == JC - 1),
                )
            nc.vector.tensor_scalar_add(kT[0][:, 0:256], pro[0][:], bks_sb[:, 0:1])
            nc.vector.tensor_scalar_add(qT[0][:, 0:256], pro[1][:], bqs_sb[:, 0:1])
            nc.vector.tensor_scalar_add(qT[0][:, 256:512], pro[2][:], bqs_sb[:, 0:1])
            # zero the v padding columns (after the prologue bias-adds so
            # they don't delay the first scores on the DVE queue)
            for s in range(SC):
                nc.vector.memset(v_sb[s][:, :, HD + 1 : 128], 0.0)

            # Filler schedules over flat iters. dlfill runs BEFORE the
            # sc(i+2) emission (only groups sc(i+2) reads -- anything before
            # a sc inflates the PE-counter threshold its exp waits on);
            # fillers runs AFTER sc(i+2), before ctx(i). Two-piece fillers
            # land on consecutive iters.
            dlfill = {}
            fillers = {}

            def dl(it, fn):
                dlfill.setdefault(it, []).append(fn)

            def fl(it, fn):
                fillers.setdefault(it, []).append(fn)

            def kg(p, g):
                return lambda: qk_group("wk", kT, bks_sb, p, g)

            def qg(p, g):
                return lambda: qk_group("wq", qT, bqs_sb, p, g)

            def place2(pk, qw, dst, bias, p, g, it):
                a, b = qk_pieces(qw, dst, bias, p, g)
                pk.setdefault(it, []).append(a)
                pk.setdefault(it + 1, []).append(b)

            # deadline groups, wall-aligned: iters 0-3 emit ONLY hidA-
            # dependent work; the first hidB-dependent instruction is
            # dlfill[4] and the first hidC-dependent one is vp(8)/kg(0,4)
            # at iters >= 14, so the PE queue never blocks on a late
            # transpose while earlier-data work is runnable behind it.
            dl(0, kg(0, 1))
            dl(4, kg(0, 2))
            dl(5, kg(0, 3))
            dl(6, qg(0, 2))
            dl(6, qg(0, 3))
            dl(14, kg(0, 4))
            dl(16, kg(0, 5))
            dl(18, kg(0, 6))
            dl(20, kg(0, 7))
            # v-projections: v0-7 (hidA/hidB) at their consuming iters;
            # v8-15 (hidC) just-in-time before the k8-15 passes
            for s in range(8):
                fl(s, lambda s=s: v_proj(s))
            for s in range(8, SC):
                fl(s + 7, lambda s=s: v_proj(s))
            # cruise fillers as 2-piece pairs (~0.33us/iter when spaced two
            # apart), after their data exists and before their consumer's
            # sc emission
            place2(fillers, "wq", qT, bqs_sb, 0, 4, 25)
            place2(fillers, "wq", qT, bqs_sb, 0, 5, 27)
            for g in range(GC):
                place2(fillers, "wk", kT, bks_sb, 1, g, 29 + 4 * g)
            place2(fillers, "wq", qT, bqs_sb, 0, 6, 39)
            place2(fillers, "wq", qT, bqs_sb, 0, 7, 43)
            place2(fillers, "wq", qT, bqs_sb, 1, 0, 59)
            qa, qb_ = qk_pieces("wq", qT, bqs_sb, 1, 1)
            fl(61, qa)
            dl(62, qb_)
            place2(fillers, "wq", qT, bqs_sb, 1, 2, 65)
            place2(fillers, "wq", qT, bqs_sb, 1, 3, 69)
            place2(fillers, "wq", qT, bqs_sb, 1, 4, 81)
            place2(fillers, "wq", qT, bqs_sb, 1, 5, 85)
            place2(fillers, "wq", qT, bqs_sb, 1, 6, 97)
            place2(fillers, "wq", qT, bqs_sb, 1, 7, 101)

            # sc emission schedule: lead-2 except across the hidB wall,
            # where sc(4..7) wait for the iter-4/5 deadline groups
            EMITS = {i: [i + 2] for i in range(NIT - 2)}
            EMITS[2] = []
            EMITS[3] = []
            EMITS[4] = [4, 5]
            EMITS[5] = [6, 7]
            EMITS[6] = [8]
            EMITS[7] = [9]

            sts = {}

            def emit_scores(i):
                pair, qb, k = ITERS[i]
                st = scps.tile([128, 2 * QB], F32, tag="sc", name="sc_t")
                qs = qb * QB
                # adjacent emission, opposite row groups -> the PE runs
                # these two K=64 matmuls concurrently
                nc.tensor.matmul(
                    st[:, 0:QB],
                    kT[pair][0:64, k * 128 : (k + 1) * 128],
                    qT[pair][0:64, qs : qs + QB],
                    start=True,
                    stop=True,
                )
                nc.tensor.matmul(
                    st[:, QB : 2 * QB],
                    kT[pair][64:128, k * 128 : (k + 1) * 128],
                    qT[pair][64:128, qs : qs + QB],
                    start=True,
                    stop=True,
                )
                sts[i] = st

            emit_scores(0)
            emit_scores(1)
            ctxs = {}
            for i in range(NIT):
                pair, qb, k = ITERS[i]
                h0, h1 = 2 * pair, 2 * pair + 1
                split = (pair, qb) in SPLIT_QBS
                if k == 0 or (split and k == 8):
                    ctxs[0] = ctxps.tile([128, QB], F32, tag="ctx0", name="ctx0")
                    ctxs[1] = ctxps.tile([128, QB], F32, tag="ctx1", name="ctx1")
                for fn in dlfill.get(i, ()):
                    fn()
                for j in EMITS.get(i, ()):
                    emit_scores(j)
                st = sts.pop(i)
                et = etp.tile([128, 2 * QB], BF16, tag="et", name="et_t")
                nc.scalar.activation(
                    et[:], st[:], EXP, bias=mask_sb[:, k : k + 1], scale=1.0
                )
                for fn in fillers.get(i, ()):
                    fn()
                cstart = k == 0 or (split and k == 8)
                cstop = k == SC - 1 or (split and k == 7)
                nc.tensor.matmul(
                    ctxs[0][:], v_sb[k][:, h0, :], et[:, 0:QB],
                    start=cstart, stop=cstop,
                )
                nc.tensor.matmul(
                    ctxs[1][:], v_sb[k][:, h1, :], et[:, QB : 2 * QB],
                    start=cstart, stop=cstop,
                )
                if split and k == 7:
                    # stage the k0-7 partial context to SBUF f32
                    for hh in (0, 1):
                        nc.vector.tensor_copy(pctx[(qb, hh)][:], ctxs[hh][:])
                if k == SC - 1:
                    # finalize: (merge +) copy ctx to SBUF bf16 NOW (frees
                    # the ctx banks for the next qb), then defer the
                    # transpose/normalize/DMA work into the next iters'
                    # filler slots (using idle score-pool PSUM) so the
                    # boundary doesn't burst-stall the exp stream
                    ctss = {}
                    for hh in (0, 1):
                        cts = ctsp.tile([128, QB], BF16, tag="cts", name="cts_t")
                        if split:
                            nc.vector.tensor_add(
                                cts[:], ctxs[hh][:], pctx[(qb, hh)][:]
                            )
                        else:
                            nc.vector.tensor_copy(cts[:], ctxs[hh][:])
                        ctss[hh] = cts

                    def tp_one(ci, hh, pool, tag, pair=pair, qb=qb, ctss=ctss):
                        h = 2 * pair + hh
                        tp = pool.tile([128, 128], BF16, tag=tag, name="tp_t")
                        nc.tensor.transpose(
                            tp[:],
                            ctss[hh][:, ci * 128 : (ci + 1) * 128],
                            ident_bf[:],
                        )
                        rc = rcp.tile([128, 1], F32, tag="rc", name="rc_t")
                        nc.vector.reciprocal(rc[:], tp[:, HD : HD + 1])
                        qc = qb * (QB // 128) + ci
                        nc.vector.tensor_scalar_mul(
                            out_sb[qc][:, h * HD : (h + 1) * HD],
                            tp[:, 0:HD],
                            rc[:],
                        )

                    def out_dma(ci, pair=pair, qb=qb, queue=nc.sync):
                        qc = qb * (QB // 128) + ci
                        queue.dma_start(
                            out[
                                qc * 128 : (qc + 1) * 128,
                                pair * 128 : (pair + 1) * 128,
                            ],
                            out_sb[qc][:, pair * 128 : (pair + 1) * 128],
                        )

                    if i == NIT - 1:
                        slots = [
                            (ctxps, "ctx0"),
                            (ctxps, "ctx1"),
                            (scps, "sc"),
                            (scps, "sc"),
                        ]
                        nt = 0
                        for ci in range(QB // 128):
                            for hh in (0, 1):
                                tp_one(ci, hh, *slots[nt % 4])
                                nt += 1
                            out_dma(ci, queue=(nc.sync if ci % 2 == 0 else nc.scalar))
                    else:
                        for ci in range(QB // 128):
                            def deferred(ci=ci, last=(ci == QB // 128 - 1)):
                                tp_one(ci, 0, scps, "sc")
                                tp_one(ci, 1, scps, "sc")
                                if last:
                                    for cj in range(QB // 128):
                                        out_dma(cj)
                            fillers.setdefault(i + 1 + ci, []).append(deferred)

    nc.compile()
    return nc


def make_in_maps(hidden_states, attention_mask, Wq, bq, Wk, bk, Wv, bv):
    hidden_states = np.asarray(hidden_states, dtype=np.float32)
    attention_mask = np.asarray(attention_mask, dtype=np.float32)
    Wq = np.asarray(Wq, dtype=np.float32)
    bq = np.asarray(bq, dtype=np.float32)
    Wk = np.asarray(Wk, dtype=np.float32)
    bk = np.asarray(bk, dtype=np.float32)
    Wv = np.asarray(Wv, dtype=np.float32)
    bv = np.asarray(bv, dtype=np.float32)
    bf = ml_dtypes.bfloat16

    def warr(w):
        # [H, DPC] -> [128, JC, DPC]: partition-major, contiguous DMA
        return w.reshape(JC, 128, DPC).transpose(1, 0, 2)

    in_maps = []
    for c in range(NCORES):
        b = c // 4
        g = c % 4
        rows = slice(g * DPC, (g + 1) * DPC)
        wkqv = np.stack(
            [
                warr(Wk[rows, :].T),
                warr((Wq[rows, :] * 0.125).T),
                warr(Wv[rows, :].T),
            ],
            axis=1,
        )
        consts = np.concatenate(
            [
                attention_mask[b, 0, 0, :].reshape(SC, 128).T,
                (bq[rows] * 0.125).reshape(2, 128).T,
                bk[rows].reshape(2, 128).T,
            ],
            axis=1,
        )
        in_maps.append(
            {
                "hidb": np.ascontiguousarray(hidden_states[b]).astype(bf),
                "wkqv": np.ascontiguousarray(wkqv).astype(bf),
                "consts": np.ascontiguousarray(consts.astype(np.float32)),
                "bvs": np.ascontiguousarray(bv[rows].reshape(1, DPC)).astype(bf),
            }
        )
    return in_maps


def gather(results):
    full = np.empty((B, S, H), dtype=np.float32)
    for c in range(NCORES):
        b = c // 4
        g = c % 4
        full[b, :, g * DPC : (g + 1) * DPC] = results[c]["out"]
    return full


_NC = None


def kernel(hidden_states, attention_mask, Wq, bq, Wk, bk, Wv, bv, **run_kwargs):
    global _NC
    if _NC is None:
        _NC = build()
    in_maps = make_in_maps(hidden_states, attention_mask, Wq, bq, Wk, bk, Wv, bv)
    res = run_bass_kernel_spmd(_NC, in_maps, core_ids=list(range(NCORES)), **run_kwargs)
    out = gather(res.results)
    if run_kwargs:
        kernel.last_result = res
    return out


# revision 48
# speedup vs baseline: 1.2696x; 1.0148x over previous
"""BertSelfAttention (B=2, S=2048, H=1024, 16 heads x 64) on 8 TRN2 NeuronCores.

Sharding: data parallel on batch (4 cores per batch) x tensor parallel on
heads (4 heads per core). No cross-core comms; each core computes
out[b, :, 256*g:256*(g+1)] for its head group g.

v10 design notes (all measured on HW traces):
- Every DMA around an X-bar transpose serializes on the previous DMA's
  completion (X-bar quiescence), so startup is ONE serial chain on the sync
  queue ordered by consumption: consts | wk+wq | bvs | T(s0-511) | wv |
  T(s512-1023) | T(s1024-2047). Weights are host-pre-arranged so their DMA
  is contiguous per partition.
- Tile deps compile to monotonic per-engine counters: exp(i) waits the PE
  counter at its sc(i) emission, so anything emitted before a sc inflates
  the exp critical path. Deadline projections (dlfill) are the only work
  emitted before sc(i+2); all other fillers go after.
- The PE p-state reaches full clock only after ~3us of continuous busy; a
  discarded warmup matmul chain covers the DMA-chain wait.
- Iteration order interleaves qb0/qb1 first halves so the ACT exp stream
  has work while the PE grinds v-projections, and the k>=8 data (last
  transposes) arrives before anything needs it. qb0/qb1 k0-7 context is
  staged to SBUF f32 and merged in the k8-15 pass.
- Score PSUM pool has 3 slots (lead-2 emission); projection PSUM shares the
  same slots via tag-sharing; output transposes reuse the two ctx banks
  (plus idle score slots on the final drain). 8 PSUM banks exactly.

Per-core pipeline:
  A) hiddenT via 24 serial X-bar transposes ([512,128]x16 + [1024,128]x8)
  B) kT/qT [128(d of pair), 2048(s)] bf16 (1/8 scale + bias folded) via
     256-col groups; V [128(s), 4heads, 65] bf16 with ones column
  C) per (pair, qb, k): scoresT h0|h1 packed -> one exp [128,1024] (mask
     bias) -> bf16 et, ctxT[65, 512] += v_ext.T @ et per head
  D) PE-transpose ctxT (bf16) -> [q, 65], DVE reciprocal + scale, DMA out
     per (pair, qb) half-block during the stream
"""

import ml_dtypes
import numpy as np

import concourse.bass as bass
import concourse.tile as tile
from concourse import bacc, mybir
from concourse.bass_utils import run_bass_kernel_spmd
from concourse.masks import make_identity

F32 = mybir.dt.float32
BF16 = mybir.dt.bfloat16
EXP = mybir.ActivationFunctionType.Exp

B, S, H = 2, 2048, 1024
NH, HD = 16, 64
NCORES = 8
HPC = 4  # heads per core
DPC = HPC * HD  # 256 output dims per core
SC = S // 128  # 16 s/k chunks
JC = H // 128  # 8 contraction chunks
QB = 512  # q block in attention inner loop
NQB = S // QB  # 4
GC = 8  # 256-col projection groups per (w, pair)

# flat iteration order: qb0/qb1 first halves interleaved into the
# v-projection grind, second halves after the late transposes land,
# then qb2/qb3 and pair 1 straight
ITERS = (
    [(0, 0, k) for k in range(8)]
    + [(0, 1, k) for k in range(8)]
    + [(0, 0, k) for k in range(8, SC)]
    + [(0, 1, k) for k in range(8, SC)]
    + [(0, 2, k) for k in range(SC)]
    + [(0, 3, k) for k in range(SC)]
    + [(1, qb, k) for qb in range(NQB) for k in range(SC)]
)
NIT = len(ITERS)  # 128
SPLIT_QBS = {(0, 0), (0, 1)}  # qbs whose k0-7 context is staged and merged


def build():
    nc = bacc.Bacc(
        "TRN2",
        target_bir_lowering=False,
        debug=False,
        enable_asserts=False,
        num_devices=NCORES,
    )
    hidb = nc.dram_tensor("hidb", [S, H], BF16, kind="ExternalInput").ap()
    wkq = nc.dram_tensor("wkq", [128, 2, JC, DPC], BF16, kind="ExternalInput").ap()
    wvd = nc.dram_tensor("wvd", [128, JC, DPC], BF16, kind="ExternalInput").ap()
    # mask | bqs | bks packed into one [128, SC+4] f32 tensor
    consts = nc.dram_tensor("consts", [128, SC + 4], F32, kind="ExternalInput").ap()
    bvs = nc.dram_tensor("bvs", [1, DPC], BF16, kind="ExternalInput").ap()
    out = nc.dram_tensor("out", [S, DPC], F32, kind="ExternalOutput").ap()

    with tile.TileContext(nc) as tc:
        with (
            tc.tile_pool(name="persist", bufs=1) as persist,
            tc.tile_pool(name="etp", bufs=8) as etp,
            tc.tile_pool(name="ctsp", bufs=2) as ctsp,
            tc.tile_pool(name="tpsb", bufs=4) as tpsb,
            tc.tile_pool(name="rcp", bufs=4) as rcp,
            tc.tile_pool(name="scps", bufs=3, space="PSUM") as scps,
            tc.tile_pool(name="ctxps", bufs=1, space="PSUM") as ctxps,
        ):
            # dummy operands for the PE warmup chain
            dummyw = persist.tile([128, 128], BF16, tag="dummyw")
            nc.vector.memset(dummyw[:], 0.0)
            dummy = persist.tile([128, 512], BF16, tag="dummy")
            nc.vector.memset(dummy[:], 0.0)

            # startup DMA chain on the sync queue (see module docstring)
            consts_sb = persist.tile([128, SC + 4], F32, tag="consts")
            nc.sync.dma_start(consts_sb[:], consts)
            mask_sb = consts_sb[:, 0:SC]
            bqs_sb = consts_sb[:, SC : SC + 2]
            bks_sb = consts_sb[:, SC + 2 : SC + 4]
            wkq_t = persist.tile([128, 2, JC, DPC], BF16, tag="wkq", name="w_kq")
            nc.sync.dma_start(wkq_t[:], wkq)
            w_sb = {"wk": wkq_t[:, 0], "wq": wkq_t[:, 1]}
            bvs_sb = persist.tile([1, DPC], BF16, tag="bvs")
            nc.sync.dma_start(bvs_sb[:], bvs)

            hidA = [
                persist.tile([128, 512], BF16, tag=f"hA{j}", name=f"hA{j}")
                for j in range(JC)
            ]
            hidB = [
                persist.tile([128, 512], BF16, tag=f"hB{j}", name=f"hB{j}")
                for j in range(JC)
            ]
            hidC = [
                persist.tile([128, 1024], BF16, tag=f"hC{j}", name=f"hC{j}")
                for j in range(JC)
            ]
            for j in range(JC):
                nc.sync.dma_start_transpose(
                    out=hidA[j][:], in_=hidb[0:512, j * 128 : (j + 1) * 128]
                )
            wv_t = persist.tile([128, JC, DPC], BF16, tag="wv", name="w_wv")
            nc.sync.dma_start(wv_t[:], wvd)
            w_sb["wv"] = wv_t
            for j in range(JC):
                nc.sync.dma_start_transpose(
                    out=hidB[j][:], in_=hidb[512:1024, j * 128 : (j + 1) * 128]
                )
            for j in range(JC):
                nc.sync.dma_start_transpose(
                    out=hidC[j][:], in_=hidb[1024:S, j * 128 : (j + 1) * 128]
                )

            ones1_f = persist.tile([1, 128], F32, tag="ones1f")
            nc.vector.memset(ones1_f[:], 1.0)
            # warm the ACT exp table during startup
            warm = persist.tile([1, 1], F32, tag="warm")
            nc.scalar.activation(warm[:], ones1_f[:, 0:1], EXP)



            ident = persist.tile([128, 128], F32, tag="ident")
            make_identity(nc, ident[:])
            ident_bf = persist.tile([128, 128], BF16, tag="identbf")
            nc.vector.tensor_copy(ident_bf[:], ident[:])
            ones1 = persist.tile([1, 128], BF16, tag="ones1")
            nc.vector.tensor_copy(ones1[:], ones1_f[:])
            ones4_f = persist.tile([128, HPC], F32, tag="ones4f")
            nc.vector.memset(ones4_f[:], 1.0)

            def hid_g(g, j):
                # 256-wide column group g of hiddenT chunk j (g = s//256)
                if g < 2:
                    return hidA[j][:, (g % 2) * 256 : (g % 2 + 1) * 256]
                if g < 4:
                    return hidB[j][:, (g % 2) * 256 : (g % 2 + 1) * 256]
                return hidC[j][:, (g - 4) * 256 : (g - 3) * 256]

            def hid_s(s, j):
                # 128-wide column chunk s of hiddenT chunk j
                if s < 4:
                    return hidA[j][:, s * 128 : (s + 1) * 128]
                if s < 8:
                    return hidB[j][:, (s - 4) * 128 : (s - 3) * 128]
                return hidC[j][:, (s - 8) * 128 : (s - 7) * 128]

            # persistent activations
            qT = [
                persist.tile([128, S], BF16, tag=f"qT{p}", name=f"qT{p}")
                for p in range(2)
            ]
            kT = [
                persist.tile([128, S], BF16, tag=f"kT{p}", name=f"kT{p}")
                for p in range(2)
            ]
            # v padded to 128 columns per head (cols 65-127 zero) so the ctx
            # matmul fills all 128 PSUM partitions -- the X-bar output
            # transpose then moves a fully-written [128,128] block
            v_sb = [
                persist.tile([128, HPC, 128], BF16, tag=f"v{s}", name=f"v{s}")
                for s in range(SC)
            ]
            out_sb = [
                persist.tile([128, DPC], F32, tag=f"o{s}", name=f"o{s}")
                for s in range(SC)
            ]
            # staged k0-7 partial contexts for the split qbs: (qb, hh)
            pctx = {
                (qb, hh): persist.tile(
                    [128, QB], F32, tag=f"pc{qb}{hh}", name=f"pc{qb}{hh}"
                )
                for qb in (0, 1)
                for hh in (0, 1)
            }

            # PE warmup: ramp the p-state (full clock needs 3us continuous
            # busy) while the DMA chain delivers. Results are discarded.
            wps = scps.tile([128, 512], F32, tag="sc", name="warmup_ps")
            for _ in range(28):
                nc.tensor.matmul(wps[:], dummyw[:], dummy[:], start=True, stop=True)

            def qk_group(wname, dst, bias, p, g):
                # one 256-wide output group of the qT/kT projection
                ps = scps.tile([128, 256], F32, tag="sc", name="projps_t")
                for j in range(JC):
                    nc.tensor.matmul(
                        ps[:],
                        w_sb[wname][:, j, p * 128 : (p + 1) * 128],
                        hid_g(g, j),
                        start=(j == 0),
                        stop=(j == JC - 1),
                    )
                nc.vector.tensor_scalar_add(
                    dst[p][:, g * 256 : (g + 1) * 256], ps[:], bias[:, p : p + 1]
                )

            def qk_pieces(wname, dst, bias, p, g):
                # the same group as two 4-chunk pieces for smooth filling
                box = {}

                def piece_a():
                    ps = scps.tile([128, 256], F32, tag="sc", name="projps_t")
                    box["ps"] = ps
                    for j in range(4):
                        nc.tensor.matmul(
                            ps[:],
                            w_sb[wname][:, j, p * 128 : (p + 1) * 128],
                            hid_g(g, j),
                            start=(j == 0),
                            stop=False,
                        )

                def piece_b():
                    ps = box["ps"]
                    for j in range(4, JC):
                        nc.tensor.matmul(
                            ps[:],
                            w_sb[wname][:, j, p * 128 : (p + 1) * 128],
                            hid_g(g, j),
                            start=False,
                            stop=(j == JC - 1),
                        )
                    nc.vector.tensor_scalar_add(
                        dst[p][:, g * 256 : (g + 1) * 256], ps[:], bias[:, p : p + 1]
                    )

                return piece_a, piece_b

            def v_proj(s):
                ps = scps.tile([128, DPC], F32, tag="sc", name="vps_t")
                for j in range(JC):
                    nc.tensor.matmul(
                        ps[:],
                        hid_s(s, j),
                        w_sb["wv"][:, j, :],
                        start=(j == 0),
                        stop=False,
                    )
                nc.tensor.matmul(ps[:], ones1[:], bvs_sb[:], start=False, stop=True)
                ps3 = ps.rearrange("p (h c) -> p h c", h=HPC)
                nc.vector.tensor_copy(v_sb[s][:, :, 0:HD], ps3[:])
                nc.vector.tensor_copy(
                    v_sb[s][:, :, HD : HD + 1],
                    ones4_f[:].rearrange("p (h o) -> p h o", o=1),
                )

            # prologue projections, j-interleaved so each 8-matmul batch
            # tracks the hidA transpose staircase just-in-time:
            # kT[0] cols 0-255, qT[0] cols 0-511
            pro = [
                scps.tile([128, 256], F32, tag="sc", name=f"props{n}")
                for n in range(3)
            ]
            for j in range(JC):
                nc.tensor.matmul(
                    pro[0][:], w_sb["wk"][:, j, 0:128], hid_g(0, j),
                    start=(j == 0), stop=(j == JC - 1),
                )
                nc.tensor.matmul(
                    pro[1][:], w_sb["wq"][:, j, 0:128], hid_g(0, j),
                    start=(j == 0), stop=(j == JC - 1),
                )
                nc.tensor.matmul(
                    pro[2][:], w_sb["wq"][:, j, 0:128], hid_g(1, j),
                    start=(j == 0), stop=(j == JC - 1),
                )
            nc.vector.tensor_scalar_add(kT[0][:, 0:256], pro[0][:], bks_sb[:, 0:1])
            nc.vector.tensor_scalar_add(qT[0][:, 0:256], pro[1][:], bqs_sb[:, 0:1])
            nc.vector.tensor_scalar_add(qT[0][:, 256:512], pro[2][:], bqs_sb[:, 0:1])
            # zero the v padding columns (after the prologue bias-adds so
            # they don't delay the first scores on the DVE queue)
            for s in range(SC):
                nc.vector.memset(v_sb[s][:, :, HD + 1 : 128], 0.0)

            # Filler schedules over flat iters. dlfill runs BEFORE the
            # sc(i+2) emission (only groups sc(i+2) reads -- anything before
            # a sc inflates the PE-counter threshold its exp waits on);
            # fillers runs AFTER sc(i+2), before ctx(i). Two-piece fillers
            # land on consecutive iters.
            dlfill = {}
            fillers = {}

            def dl(it, fn):
                dlfill.setdefault(it, []).append(fn)

            def fl(it, fn):
                fillers.setdefault(it, []).append(fn)

            def kg(p, g):
                return lambda: qk_group("wk", kT, bks_sb, p, g)

            def qg(p, g):
                return lambda: qk_group("wq", qT, bqs_sb, p, g)

            def place2(pk, qw, dst, bias, p, g, it):
                a, b = qk_pieces(qw, dst, bias, p, g)
                pk.setdefault(it, []).append(a)
                pk.setdefault(it + 1, []).append(b)

            # deadline groups, wall-aligned: iters 0-3 emit ONLY hidA-
            # dependent work; the first hidB-dependent instruction is
            # dlfill[4] and the first hidC-dependent one is vp(8)/kg(0,4)
            # at iters >= 14, so the PE queue never blocks on a late
            # transpose while earlier-data work is runnable behind it.
            dl(0, kg(0, 1))
            dl(4, kg(0, 2))
            dl(5, kg(0, 3))
            dl(6, qg(0, 2))
            dl(6, qg(0, 3))
            # hidC-dep kT groups split so the deadline slot (gating the
            # exp of the sc that reads them) carries only half a group
            for g in range(4, GC):
                a, b = qk_pieces("wk", kT, bks_sb, 0, g)
                fl(2 * g + 5, a)
                dl(2 * g + 6, b)
            # v-projections: v0-7 (hidA/hidB) at their consuming iters;
            # v8-15 (hidC) just-in-time before the k8-15 passes
            for s in range(8):
                fl(s, lambda s=s: v_proj(s))
            for s in range(8, SC):
                fl(s + 7, lambda s=s: v_proj(s))
            # cruise fillers as 2-piece pairs (~0.33us/iter when spaced two
            # apart), after their data exists and before their consumer's
            # sc emission
            place2(fillers, "wq", qT, bqs_sb, 0, 4, 25)
            place2(fillers, "wq", qT, bqs_sb, 0, 5, 27)
            for g in range(GC):
                place2(fillers, "wk", kT, bks_sb, 1, g, 29 + 4 * g)
            place2(fillers, "wq", qT, bqs_sb, 0, 6, 39)
            place2(fillers, "wq", qT, bqs_sb, 0, 7, 43)
            place2(fillers, "wq", qT, bqs_sb, 1, 0, 59)
            qa, qb_ = qk_pieces("wq", qT, bqs_sb, 1, 1)
            fl(61, qa)
            dl(62, qb_)
            place2(fillers, "wq", qT, bqs_sb, 1, 2, 65)
            place2(fillers, "wq", qT, bqs_sb, 1, 3, 69)
            place2(fillers, "wq", qT, bqs_sb, 1, 4, 81)
            place2(fillers, "wq", qT, bqs_sb, 1, 5, 85)
            place2(fillers, "wq", qT, bqs_sb, 1, 6, 97)
            place2(fillers, "wq", qT, bqs_sb, 1, 7, 101)

            # sc emission schedule: lead-2 except across the hidB wall,
            # where sc(4..7) wait for the iter-4/5 deadline groups
            EMITS = {i: [i + 2] for i in range(NIT - 2)}
            EMITS[2] = []
            EMITS[3] = []
            EMITS[4] = [4, 5]
            EMITS[5] = [6, 7]
            EMITS[6] = [8]
            EMITS[7] = [9]

            sts = {}

            def emit_scores(i):
                pair, qb, k = ITERS[i]
                st = scps.tile([128, 2 * QB], F32, tag="sc", name="sc_t")
                qs = qb * QB
                # adjacent emission, opposite row groups -> the PE runs
                # these two K=64 matmuls concurrently
                nc.tensor.matmul(
                    st[:, 0:QB],
                    kT[pair][0:64, k * 128 : (k + 1) * 128],
                    qT[pair][0:64, qs : qs + QB],
                    start=True,
                    stop=True,
                )
                nc.tensor.matmul(
                    st[:, QB : 2 * QB],
                    kT[pair][64:128, k * 128 : (k + 1) * 128],
                    qT[pair][64:128, qs : qs + QB],
                    start=True,
                    stop=True,
                )
                sts[i] = st

            emit_scores(0)
            emit_scores(1)
            ctxs = {}
            for i in range(NIT):
                pair, qb, k = ITERS[i]
                h0, h1 = 2 * pair, 2 * pair + 1
                split = (pair, qb) in SPLIT_QBS
                if k == 0 or (split and k == 8):
                    ctxs[0] = ctxps.tile([128, QB], F32, tag="ctx0", name="ctx0")
                    ctxs[1] = ctxps.tile([128, QB], F32, tag="ctx1", name="ctx1")
                for fn in dlfill.get(i, ()):
                    fn()
                for j in EMITS.get(i, ()):
                    emit_scores(j)
                st = sts.pop(i)
                et = etp.tile([128, 2 * QB], BF16, tag="et", name="et_t")
                nc.scalar.activation(
                    et[:], st[:], EXP, bias=mask_sb[:, k : k + 1], scale=1.0
                )
                for fn in fillers.get(i, ()):
                    fn()
                cstart = k == 0 or (split and k == 8)
                cstop = k == SC - 1 or (split and k == 7)
                nc.tensor.matmul(
                    ctxs[0][:], v_sb[k][:, h0, :], et[:, 0:QB],
                    start=cstart, stop=cstop,
                )
                nc.tensor.matmul(
                    ctxs[1][:], v_sb[k][:, h1, :], et[:, QB : 2 * QB],
                    start=cstart, stop=cstop,
                )
                if split and k == 7:
                    # stage the k0-7 partial context to SBUF f32
                    for hh in (0, 1):
                        nc.vector.tensor_copy(pctx[(qb, hh)][:], ctxs[hh][:])
                if k == SC - 1:
                    # finalize: (merge +) copy ctx to SBUF bf16 NOW (frees
                    # the ctx banks for the next qb), then defer the
                    # transpose/normalize/DMA work into the next iters'
                    # filler slots (using idle score-pool PSUM) so the
                    # boundary doesn't burst-stall the exp stream
                    ctss = {}
                    for hh in (0, 1):
                        cts = ctsp.tile([128, QB], BF16, tag="cts", name="cts_t")
                        if split:
                            nc.vector.tensor_add(
                                cts[:], ctxs[hh][:], pctx[(qb, hh)][:]
                            )
                        else:
                            nc.vector.tensor_copy(cts[:], ctxs[hh][:])
                        ctss[hh] = cts

                    def tp_one(ci, hh, pool, tag, pair=pair, qb=qb, ctss=ctss):
                        h = 2 * pair + hh
                        tp = pool.tile([128, 128], BF16, tag=tag, name="tp_t")
                        nc.tensor.transpose(
                            tp[:],
                            ctss[hh][:, ci * 128 : (ci + 1) * 128],
                            ident_bf[:],
                        )
                        rc = rcp.tile([128, 1], F32, tag="rc", name="rc_t")
                        nc.vector.reciprocal(rc[:], tp[:, HD : HD + 1])
                        qc = qb * (QB // 128) + ci
                        nc.vector.tensor_scalar_mul(
                            out_sb[qc][:, h * HD : (h + 1) * HD],
                            tp[:, 0:HD],
                            rc[:],
                        )

                    def out_dma(ci, pair=pair, qb=qb, queue=nc.sync):
                        qc = qb * (QB // 128) + ci
                        queue.dma_start(
                            out[
                                qc * 128 : (qc + 1) * 128,
                                pair * 128 : (pair + 1) * 128,
                            ],
                            out_sb[qc][:, pair * 128 : (pair + 1) * 128],
                        )

                    if i == NIT - 1:
                        slots = [
                            (ctxps, "ctx0"),
                            (ctxps, "ctx1"),
                            (scps, "sc"),
                            (scps, "sc"),
                        ]
                        nt = 0
                        for ci in range(QB // 128):
                            for hh in (0, 1):
                                tp_one(ci, hh, *slots[nt % 4])
                                nt += 1
                            out_dma(ci, queue=(nc.sync if ci % 2 == 0 else nc.scalar))
                    else:
                        for ci in range(QB // 128):
                            def deferred(ci=ci, last=(ci == QB // 128 - 1)):
                                tp_one(ci, 0, scps, "sc")
                                tp_one(ci, 1, scps, "sc")
                                if last:
                                    for cj in range(QB // 128):
                                        out_dma(cj)
                            fillers.setdefault(i + 1 + ci, []).append(deferred)

    nc.compile()
    return nc


def make_in_maps(hidden_states, attention_mask, Wq, bq, Wk, bk, Wv, bv):
    hidden_states = np.asarray(hidden_states, dtype=np.float32)
    attention_mask = np.asarray(attention_mask, dtype=np.float32)
    Wq = np.asarray(Wq, dtype=np.float32)
    bq = np.asarray(bq, dtype=np.float32)
    Wk = np.asarray(Wk, dtype=np.float32)
    bk = np.asarray(bk, dtype=np.float32)
    Wv = np.asarray(Wv, dtype=np.float32)
    bv = np.asarray(bv, dtype=np.float32)
    bf = ml_dtypes.bfloat16

    def warr(w):
        # [H, DPC] -> [128, JC, DPC]: partition-major, contiguous DMA
        return w.reshape(JC, 128, DPC).transpose(1, 0, 2)

    in_maps = []
    for c in range(NCORES):
        b = c // 4
        g = c % 4
        rows = slice(g * DPC, (g + 1) * DPC)
        wkq = np.stack(
            [warr(Wk[rows, :].T), warr((Wq[rows, :] * 0.125).T)], axis=1
        )
        consts = np.concatenate(
            [
                attention_mask[b, 0, 0, :].reshape(SC, 128).T,
                (bq[rows] * 0.125).reshape(2, 128).T,
                bk[rows].reshape(2, 128).T,
            ],
            axis=1,
        )
        in_maps.append(
            {
                "hidb": np.ascontiguousarray(hidden_states[b]).astype(bf),
                "wkq": np.ascontiguousarray(wkq).astype(bf),
                "wvd": np.ascontiguousarray(warr(Wv[rows, :].T)).astype(bf),
                "consts": np.ascontiguousarray(consts.astype(np.float32)),
                "bvs": np.ascontiguousarray(bv[rows].reshape(1, DPC)).astype(bf),
            }
        )
    return in_maps


def gather(results):
    full = np.empty((B, S, H), dtype=np.float32)
    for c in range(NCORES):
        b = c // 4
        g = c % 4
        full[b, :, g * DPC : (g + 1) * DPC] = results[c]["out"]
    return full


_NC = None


def kernel(hidden_states, attention_mask, Wq, bq, Wk, bk, Wv, bv, **run_kwargs):
    global _NC
    if _NC is None:
        _NC = build()
    in_maps = make_in_maps(hidden_states, attention_mask, Wq, bq, Wk, bk, Wv, bv)
    res = run_bass_kernel_spmd(_NC, in_maps, core_ids=list(range(NCORES)), **run_kwargs)
    out = gather(res.results)
    if run_kwargs:
        kernel.last_result = res
    return out
